# revision 30
# baseline (speedup 1.0000x reference)
"""DyRep event-batch kernel for 8 trn2 NeuronCores.

Strategy (data-parallel over the event batch B=256, 32 events/core):
  - sigmoid is monotone, so
        max_n(where(mask, sigmoid(q*h), -inf)) == sigmoid(max_n masked q*h),
    and q = exp(s)/sum factors so only max_n(exp(s)*h) is needed per row
    (the 1/denominator is applied after the max).
  - S = A/deg means S[p,n] > 0 iff A[p,n] > 0, so only S rows are read
    (A is never shipped).  Neighbors (~32 of 8192 per row) are extracted
    on-device with windowed top-8 (nc.vector.max/max_index) over 256-wide
    windows; a tiny descending positional perturbation (built on-chip via
    iota) makes all values distinct so tie-breaking is well defined.
  - All gathers use the gpsimd dma_gather ucode op (indirect_dma_start's
    dynamic-AP path mis-executes on hardware).  Its wrapped int16 index
    lists are host-packed for the S/z gathers; the data-dependent h-row
    list is built on-device with PE transposes (wrapped = transpose of the
    list viewed [1024, 16], done as 16-column slice transposes).
  - Candidate h-rows come from an on-device h_all = z @ W_h^T + b table
    (padded to 64 cols = 256B rows); invalid slots point at a -1e30 row so
    one mul + one max-reduce finishes the aggregation.
  - Lambda = psi_k * softplus(g_psi) computed as ln(1+exp(x)) on ACT.
"""

import numpy as np

import concourse.bacc as bacc
import concourse.bass as bass
import concourse.mybir as mybir
import concourse.tile as tile
from concourse.bass_utils import run_bass_kernel_spmd
from concourse.masks import make_identity

N, H, B = 8192, 32, 256
HP = 64                   # padded h_all row (256B)
NCORES = 8
BE = B // NCORES          # events per core
R = 2 * BE                # rows (event-sides) per core
WIN = 256                 # top-8 window width
NWIN = 4096 // WIN        # windows per half-row (16)
SLOTS = 8 * NWIN          # candidate slots per half-row partition (128)
NCHUNK = 4                # slot chunks for pipelining
CS = SLOTS // NCHUNK      # slots per chunk (32)
EPS = 2.0 ** -20
NEG = -1.0e30
F32 = mybir.dt.float32
I16 = mybir.dt.int16
I32 = mybir.dt.int32
U32 = mybir.dt.uint32

_prog_cache = {}


def _build_program(compile_=True):
    nc = bacc.Bacc(num_swdge_queues=4)

    # ---- external inputs (per-core) ----
    S_d = nc.dram_tensor("S", [N, N], F32, kind="ExternalInput")
    zcw_d = nc.dram_tensor("zcw", [H + 1, N + HP], F32, kind="ExternalInput")
    z64_d = nc.dram_tensor("z64", [N, HP], F32, kind="ExternalInput")
    sidx_d = nc.dram_tensor("sidx", [128, 8], I16, kind="ExternalInput")
    tidx_d = nc.dram_tensor("tidx", [128, 4], I16, kind="ExternalInput")
    kf_d = nc.dram_tensor("kf", [BE, 1], F32, kind="ExternalInput")
    psi_d = nc.dram_tensor("psi2", [BE, 2], F32, kind="ExternalInput")
    omb_d = nc.dram_tensor("omb2", [BE, 2], F32, kind="ExternalInput")
    tdT_d = nc.dram_tensor("tdT", [4, R], F32, kind="ExternalInput")
    wrec_d = nc.dram_tensor("WrecT_aug", [H + 1, H], F32, kind="ExternalInput")
    wstr_d = nc.dram_tensor("WstructT", [H, H], F32, kind="ExternalInput")
    wt_d = nc.dram_tensor("WtT", [4, H], F32, kind="ExternalInput")
    om_d = nc.dram_tensor("OM", [2 * H, 4], F32, kind="ExternalInput")
    sel_d = nc.dram_tensor("sel", [128, 512], F32, kind="ExternalInput")

    # ---- external outputs ----
    lam_d = nc.dram_tensor("Lam", [BE, 1], F32, kind="ExternalOutput")
    zn_d = nc.dram_tensor("zn", [R, H], F32, kind="ExternalOutput")

    # ---- internal scratch ----
    hall_d = nc.dram_tensor("hall", [N + 1, HP], F32)

    with tile.TileContext(nc) as tc:
        with (
            tc.tile_pool(name="sbuf", bufs=1) as sb,
            tc.tile_pool(name="hstage", bufs=3) as hst,
            tc.tile_pool(name="hgp", bufs=3) as hgp,
            tc.tile_pool(name="psum", bufs=2, space="PSUM") as ps,
            tc.tile_pool(name="pst", bufs=1, space="PSUM") as pst,
            tc.tile_pool(name="ptail", bufs=1, space="PSUM") as ptail,
        ):
            ident = sb.tile([128, 128], F32)
            make_identity(nc, ident[:])

            # ============ hoisted small loads + target-row gather ============
            tidx_sb = sb.tile([128, 4], I16)
            nc.sync.dma_start(tidx_sb[:], tidx_d[:])
            ztg = sb.tile([128, 1, HP], F32)
            nc.gpsimd.dma_gather(
                out_ap=ztg[:, :, :], in_ap=z64_d[:, :], idxs_ap=tidx_sb[:, :],
                num_idxs=R, num_idxs_reg=R, elem_size=HP,
            )
            ztarg = ztg[:, 0, 0:H]  # [128(use 64), 32]
            wrec_sb = sb.tile([H + 1, H], F32)
            nc.sync.dma_start(wrec_sb[:], wrec_d[:])
            wstr_sb = sb.tile([H, H], F32)
            nc.sync.dma_start(wstr_sb[:], wstr_d[:])
            wt_sb = sb.tile([4, H], F32)
            nc.sync.dma_start(wt_sb[:], wt_d[:])
            tdT_sb = sb.tile([4, R], F32)
            nc.sync.dma_start(tdT_sb[:], tdT_d[:])
            om_sb = sb.tile([2 * H, 4], F32)
            nc.sync.dma_start(om_sb[:], om_d[:])
            kf_sb = sb.tile([BE, 1], F32)
            nc.sync.dma_start(kf_sb[:], kf_d[:])
            psi_sb = sb.tile([BE, 2], F32)
            nc.sync.dma_start(psi_sb[:], psi_d[:])
            omb_sb = sb.tile([BE, 2], F32)
            nc.sync.dma_start(omb_sb[:], omb_d[:])
            sel_sb = sb.tile([128, 512], F32)
            nc.sync.dma_start(sel_sb[:], sel_d[:])
            ztT = sb.tile([H + 1, R], F32)
            nc.vector.memset(ztT[H : H + 1, :], 1.0)

            # ========== Phase A: h_all = z @ W_h^T + b (padded cols) ==========
            zc_sb = sb.tile([H + 1, N + HP], F32)
            nc.sync.dma_start(zc_sb[:], zcw_d[:])
            wh_sb = zc_sb[:, N : N + HP]

            negrow = sb.tile([1, HP], F32)
            nc.vector.memset(negrow[:], NEG)
            nc.sync.dma_start(hall_d[N : N + 1, :], negrow[:])

            for gb in range(4):
                hsb = hst.tile([128, 16 * HP], F32, tag="hsb")
                for g4 in range(4):
                    g = 4 * gb + g4
                    ph = ps.tile([128, 4 * HP], F32, tag="ph")
                    for j in range(4):
                        nt = 4 * g + j
                        nc.tensor.matmul(
                            out=ph[:, HP * j : HP * j + HP],
                            lhsT=zc_sb[:, 128 * nt : 128 * nt + 128],
                            rhs=wh_sb[:],
                            start=True,
                            stop=True,
                        )
                    nc.scalar.copy(
                        hsb[:, 4 * HP * g4 : 4 * HP * (g4 + 1)], ph[:]
                    )
                nc.sync.dma_start(
                    hall_d[2048 * gb : 2048 * gb + 2048, :].rearrange(
                        "(j p) h -> p j h", j=16
                    ),
                    hsb[:].rearrange("p (j h) -> p j h", j=16),
                )

            # ================= Phase B-E: chunked S/topk/gather ==============
            sidx_sb = sb.tile([128, 8], I16)
            nc.sync.dma_start(sidx_sb[:], sidx_d[:])
            stg = sb.tile([128, 1, 4096], F32)
            st = stg[:, 0, :]  # [128, 4096]
            S_view = S_d[:, :].rearrange("n (a b) -> (n a) b", a=2)

            # perturbation ramp: distinct values within each 256-window
            pio = sb.tile([128, WIN], I32)
            nc.gpsimd.iota(pio[:], pattern=[[1, WIN]], base=0,
                           channel_multiplier=0)
            pert = sb.tile([128, WIN], F32)
            nc.scalar.activation(pert[:], pio[:],
                                 mybir.ActivationFunctionType.Copy,
                                 bias=-EPS, scale=-EPS)
            # window offsets for global index reconstruction
            wio = sb.tile([128, SLOTS], I32)
            nc.gpsimd.iota(wio[:], pattern=[[WIN, NWIN], [0, 8]], base=0,
                           channel_multiplier=0)
            woff = sb.tile([128, SLOTS], F32)
            nc.vector.tensor_copy(out=woff[:], in_=wio[:])

            mf = sb.tile([128, SLOTS], F32)
            mi = sb.tile([128, SLOTS], U32)
            ssump = sb.tile([128, NCHUNK], F32)
            em = sb.tile([128, SLOTS], F32)
            wrapped = sb.tile([128, SLOTS * 8], I16)
            hmaxp = sb.tile([128, NCHUNK * H], F32)
            CW = 4096 // NCHUNK  # S columns per chunk

            for q in range(NCHUNK):
                qs = slice(CS * q, CS * q + CS)
                cw = slice(CW * q, CW * q + CW)
                # ---- gather this chunk of all S half-rows ----
                nc.gpsimd.dma_gather(
                    out_ap=stg[:, :, cw],
                    in_ap=S_view[:, cw],
                    idxs_ap=sidx_sb[:, :],
                    num_idxs=128, num_idxs_reg=128, elem_size=CW,
                    elem_step=4096, queue_num=q % 4,
                )
                # st_chunk += pert (repeated over this chunk's windows)
                nc.gpsimd.tensor_tensor(
                    out=st[:, cw].rearrange("p (w j) -> p w j", j=WIN),
                    in0=st[:, cw].rearrange("p (w j) -> p w j", j=WIN),
                    in1=pert[:, None, :].to_broadcast([128, CW // WIN, WIN]),
                    op=mybir.AluOpType.add,
                )
                # ---- top-8 per 256-window (4 windows per chunk) ----
                for wj in range(4):
                    w = 4 * q + wj
                    sl = slice(8 * w, 8 * w + 8)
                    win = st[:, WIN * w : WIN * w + WIN]
                    nc.vector.max(out=mf[:, sl], in_=win)
                    nc.vector.max_index(out=mi[:, sl], in_max=mf[:, sl],
                                        in_values=win)
                # ---- slot math ----
                validf = sb.tile([128, CS], F32, tag="validf")
                nc.vector.tensor_scalar(
                    out=validf[:], in0=mf[:, qs], scalar1=0.0, scalar2=None,
                    op0=mybir.AluOpType.is_gt,
                )
                mif = sb.tile([128, CS], F32, tag="mif")
                nc.vector.tensor_copy(out=mif[:], in_=mi[:, qs])
                # exact S value: s = mf + (mif+1)*EPS  (pert is window-local)
                sex = sb.tile([128, CS], F32, tag="sex")
                nc.vector.tensor_scalar(
                    out=sex[:], in0=mif[:], scalar1=EPS, scalar2=EPS,
                    op0=mybir.AluOpType.mult, op1=mybir.AluOpType.add,
                )
                nc.vector.tensor_tensor(
                    out=sex[:], in0=sex[:], in1=mf[:, qs],
                    op=mybir.AluOpType.add,
                )
                exc = sb.tile([128, CS], F32, tag="exc")
                nc.scalar.activation(exc[:], sex[:],
                                     mybir.ActivationFunctionType.Exp)
                ev = sb.tile([128, CS], F32, tag="ev")
                nc.vector.tensor_tensor(out=ev[:], in0=exc[:], in1=validf[:],
                                        op=mybir.AluOpType.mult)
                nc.vector.reduce_sum(out=ssump[:, q : q + 1], in_=ev[:],
                                     axis=mybir.AxisListType.X)
                # em = ev - valid + 1   (1.0 on invalid slots)
                nc.vector.tensor_tensor(out=em[:, qs], in0=ev[:],
                                        in1=validf[:],
                                        op=mybir.AluOpType.subtract)
                nc.vector.tensor_scalar_add(em[:, qs], em[:, qs], 1.0)
                # gather index: lf = mif + woff (+4096 on half-1); invalid -> N
                lf = sb.tile([128, CS], F32, tag="lf")
                nc.vector.tensor_tensor(out=lf[:], in0=mif[:],
                                        in1=woff[:, qs],
                                        op=mybir.AluOpType.add)
                nc.vector.tensor_scalar_add(lf[64:128, :], lf[64:128, :],
                                            4096.0)
                nc.vector.tensor_scalar_add(lf[:], lf[:], float(-N))
                nc.vector.tensor_tensor(out=lf[:], in0=lf[:], in1=validf[:],
                                        op=mybir.AluOpType.mult)
                nc.vector.tensor_scalar_add(lf[:], lf[:], float(N))
                # ---- wrap lf into dma_gather's [16]-wrapped int16 layout ----
                # list[i] = lf[i%128, CS*q + i//128]; wrapped[w,f] = list[16f+w]
                # wa_a[p, b] = lf[16a + p%16, b] done as one-hot matmuls,
                # replicated across all 8 gpsimd core groups for free
                wcol = slice(CS * 8 * q, CS * 8 * q + CS * 8)
                for a in range(8):
                    j2, aa = a // 4, a % 4
                    wa_ps = pst.tile([128, CS], F32, tag="wa")
                    nc.tensor.matmul(
                        out=wa_ps[:],
                        lhsT=sel_sb[
                            64 * j2 : 64 * j2 + 64,
                            128 * aa : 128 * aa + 128,
                        ],
                        rhs=lf[64 * j2 : 64 * j2 + 64, :],
                        start=True, stop=True,
                    )
                    nc.vector.tensor_copy(
                        out=wrapped[:, :]
                        .rearrange("w (b a) -> w b a", a=8)[
                            :, CS * q : CS * q + CS, a
                        ],
                        in_=wa_ps[:],
                    )
                # ---- h gather + masked max for this chunk ----
                hg = hgp.tile([128, CS, HP], F32, tag="hg")
                nc.gpsimd.dma_gather(
                    out_ap=hg[:, :, :], in_ap=hall_d[:, :],
                    idxs_ap=wrapped[:, wcol],
                    num_idxs=128 * CS, num_idxs_reg=128 * CS, elem_size=HP,
                    single_packet=False, queue_num=(q + 1) % 4,
                )
                tm = hgp.tile([128, CS, H], F32, tag="tm")
                nc.vector.tensor_tensor(
                    out=tm[:],
                    in0=hg[:, :, 0:H],
                    in1=em[:, qs, None].to_broadcast([128, CS, H]),
                    op=mybir.AluOpType.mult,
                )
                nc.vector.tensor_reduce(
                    out=hmaxp[:, H * q : H * q + H],
                    in_=tm[:].rearrange("p s h -> p h s"),
                    axis=mybir.AxisListType.X,
                    op=mybir.AluOpType.max,
                )

            # ---- combine: denominators and final max ----
            ssum = sb.tile([128, 1], F32)
            nc.vector.reduce_sum(out=ssum[:], in_=ssump[:],
                                 axis=mybir.AxisListType.X)
            ssum_hi = sb.tile([64, 1], F32)
            nc.sync.dma_start(ssum_hi[:], ssum[64:128, :])
            den = sb.tile([64, 1], F32)
            nc.vector.tensor_tensor(out=den[:], in0=ssum[0:64, :],
                                    in1=ssum_hi[:], op=mybir.AluOpType.add)
            nc.vector.tensor_scalar_add(den[:], den[:], 1.0e-7)
            rcp = sb.tile([64, 1], F32)
            nc.vector.reciprocal(rcp[:], den[:])

            hmax = sb.tile([128, H], F32)
            nc.vector.tensor_reduce(
                out=hmax[:],
                in_=hmaxp[:].rearrange("p (q h) -> p h q", h=H),
                axis=mybir.AxisListType.X, op=mybir.AluOpType.max,
            )
            hmax_hi = sb.tile([64, H], F32)
            nc.sync.dma_start(hmax_hi[:], hmax[64:128, :])
            hs = sb.tile([64, H], F32)
            nc.vector.tensor_tensor(out=hs[:], in0=hmax[0:64, :],
                                    in1=hmax_hi[:], op=mybir.AluOpType.max)
            nc.vector.tensor_scalar_mul(hs[:], hs[:], rcp[:, 0:1])
            hstruct = sb.tile([64, H], F32)
            nc.scalar.activation(hstruct[:], hs[:],
                                 mybir.ActivationFunctionType.Sigmoid)

            # ============ Lambda pre-work (independent of h_struct) ==========
            ztT_ps = ptail.tile([H, R], F32, tag="tps")
            nc.tensor.transpose(ztT_ps[:], ztarg[0:R, :], ident[0:R, 0:R])
            nc.scalar.copy(ztT[0:H, :], ztT_ps[:])
            catT = sb.tile([2 * H, BE], F32)
            nc.scalar.copy(catT[0:H, :], ztT[0:H, 0:BE])
            nc.scalar.copy(catT[H : 2 * H, :], ztT[0:H, BE:R])
            G_ps = ptail.tile([BE, 4], F32, tag="g")
            nc.tensor.matmul(out=G_ps[:], lhsT=catT[:], rhs=om_sb[:],
                             start=True, stop=True)
            Gs = sb.tile([BE, 4], F32)
            nc.scalar.copy(Gs[:], G_ps[:])

            g0 = sb.tile([BE, 1], F32)
            g1 = sb.tile([BE, 1], F32)
            nc.vector.tensor_tensor(out=g0[:], in0=Gs[:, 0:1], in1=Gs[:, 1:2],
                                    op=mybir.AluOpType.add)
            nc.vector.tensor_scalar(out=g0[:], in0=g0[:], scalar1=0.5,
                                    scalar2=omb_sb[:, 0:1],
                                    op0=mybir.AluOpType.mult,
                                    op1=mybir.AluOpType.add)
            nc.vector.tensor_tensor(out=g1[:], in0=Gs[:, 2:3], in1=Gs[:, 3:4],
                                    op=mybir.AluOpType.add)
            nc.vector.tensor_scalar(out=g1[:], in0=g1[:], scalar1=0.5,
                                    scalar2=omb_sb[:, 1:2],
                                    op0=mybir.AluOpType.mult,
                                    op1=mybir.AluOpType.add)
            gg = sb.tile([BE, 1], F32)
            nc.vector.tensor_tensor(out=gg[:], in0=g1[:], in1=g0[:],
                                    op=mybir.AluOpType.subtract)
            nc.vector.tensor_tensor(out=gg[:], in0=gg[:], in1=kf_sb[:],
                                    op=mybir.AluOpType.mult)
            nc.vector.tensor_tensor(out=gg[:], in0=gg[:], in1=g0[:],
                                    op=mybir.AluOpType.add)
            psik = sb.tile([BE, 1], F32)
            nc.vector.tensor_tensor(out=psik[:], in0=psi_sb[:, 1:2],
                                    in1=psi_sb[:, 0:1],
                                    op=mybir.AluOpType.subtract)
            nc.vector.tensor_tensor(out=psik[:], in0=psik[:], in1=kf_sb[:],
                                    op=mybir.AluOpType.mult)
            nc.vector.tensor_tensor(out=psik[:], in0=psik[:],
                                    in1=psi_sb[:, 0:1],
                                    op=mybir.AluOpType.add)
            pden = sb.tile([BE, 1], F32)
            nc.vector.tensor_scalar_add(pden[:], psik[:], 1.0e-7)
            rp = sb.tile([BE, 1], F32)
            nc.vector.reciprocal(rp[:], pden[:])
            gp = sb.tile([BE, 1], F32)
            nc.vector.tensor_tensor(out=gp[:], in0=gg[:], in1=rp[:],
                                    op=mybir.AluOpType.mult)
            nc.vector.tensor_scalar_min(gp[:], gp[:], 75.0)
            nc.vector.tensor_scalar_max(gp[:], gp[:], -75.0)
            # softplus(x) = ln(1 + exp(x));  |x| <= 75 so exp stays in range
            spe = sb.tile([BE, 1], F32)
            nc.scalar.activation(spe[:], gp[:],
                                 mybir.ActivationFunctionType.Exp)
            sp = sb.tile([BE, 1], F32)
            nc.scalar.activation(sp[:], spe[:],
                                 mybir.ActivationFunctionType.Ln, bias=1.0)
            lam_sb = sb.tile([BE, 1], F32)
            nc.vector.tensor_tensor(out=lam_sb[:], in0=sp[:], in1=psik[:],
                                    op=mybir.AluOpType.mult)
            nc.sync.dma_start(lam_d[:], lam_sb[:])

            # ================= Phase F: z_new ================================
            tc.strict_bb_all_engine_barrier()

            hsT_ps = ptail.tile([H, R], F32, tag="tps")
            nc.tensor.transpose(hsT_ps[:], hstruct[:], ident[0:R, 0:R])
            hsT = sb.tile([H, R], F32)
            nc.scalar.copy(hsT[:], hsT_ps[:])

            zn_ps = ptail.tile([R, H], F32, tag="zn")
            nc.tensor.matmul(out=zn_ps[:], lhsT=ztT[:], rhs=wrec_sb[:],
                             start=True, stop=False)
            nc.tensor.matmul(out=zn_ps[:], lhsT=tdT_sb[:], rhs=wt_sb[:],
                             start=False, stop=False)
            nc.tensor.matmul(out=zn_ps[:], lhsT=hsT[:], rhs=wstr_sb[:],
                             start=False, stop=True)
            zn_sb = sb.tile([R, H], F32)
            nc.scalar.activation(zn_sb[:], zn_ps[:],
                                 mybir.ActivationFunctionType.Sigmoid)
            nc.sync.dma_start(zn_d[:], zn_sb[:])

    if compile_:
        nc.compile()
    return nc


def _wrap16(lst):
    """dma_gather wrapped index layout: list[i] -> [i%16, i//16], int16,
    replicated to all 8 gpsimd core groups (128 partitions)."""
    lst = np.asarray(lst, np.int16)
    n = len(lst)
    assert n % 16 == 0
    w = lst.reshape(n // 16, 16).T.copy()          # [16, n//16]
    return np.tile(w, (8, 1)).copy()               # [128, n//16]


def _pack_inputs(z, S, u_idx, v_idx, k, time_delta,
                 W_h_w, W_h_b, W_struct_w, W_struct_b,
                 W_rec_w, W_rec_b, W_t_w, W_t_b,
                 om0_w, om0_b, om1_w, om1_b, psi):
    f = np.float32
    z = np.ascontiguousarray(z, f)
    zc = np.concatenate([z.T, np.ones((1, N), f)], axis=0)
    wh_pack = np.concatenate([W_h_w.T, W_h_b[None, :]], axis=0).astype(f)
    wh_pad = np.concatenate([wh_pack, np.zeros((H + 1, HP - H), f)], axis=1)
    zcw = np.concatenate([zc, wh_pad], axis=1)
    z64 = np.concatenate([z, np.zeros((N, HP - H), f)], axis=1)

    b_total = (W_struct_b + W_rec_b + W_t_b).astype(f)
    wrec = np.concatenate([W_rec_w.T, b_total[None, :]], axis=0).astype(f)
    wstr = np.ascontiguousarray(W_struct_w.T, f)
    wt = np.ascontiguousarray(W_t_w.T, f)
    om0 = om0_w[0].astype(f)
    om1 = om1_w[0].astype(f)
    om0s = np.concatenate([om0[H:], om0[:H]])
    om1s = np.concatenate([om1[H:], om1[:H]])
    OM = np.stack([om0, om0s, om1, om1s], axis=1).astype(f)
    omb2 = np.broadcast_to(np.array([om0_b[0], om1_b[0]], f), (BE, 2)).copy()
    sel1 = np.zeros((64, 512), f)
    for aa in range(4):
        for p in range(128):
            sel1[16 * aa + p % 16, 128 * aa + p] = 1.0
    sel = np.tile(sel1, (2, 1))

    psi2 = np.broadcast_to(np.asarray(psi, f), (BE, 2)).copy()
    S = np.ascontiguousarray(S, f)

    in_maps = []
    for c in range(NCORES):
        sl = slice(c * BE, (c + 1) * BE)
        u = np.asarray(u_idx[sl], np.int64)
        v = np.asarray(v_idx[sl], np.int64)
        partner = np.concatenate([v, u])            # row r partner
        target = np.concatenate([u, v])             # row r target
        # S half-row gather list: partition p < 64 -> half 0, else half 1
        s_list = np.concatenate([2 * partner, 2 * partner + 1])
        sidx = _wrap16(s_list)
        tidx = _wrap16(target)                      # 64 idxs -> [128, 4]
        kf = np.asarray(k[sl], f)[:, None].copy()
        td = np.asarray(time_delta[sl], f)          # [BE, 2, 4]
        tdT = np.ascontiguousarray(td.transpose(2, 1, 0).reshape(4, R))
        in_maps.append(dict(
            S=S, zcw=zcw, z64=z64, sidx=sidx, tidx=tidx,
            kf=kf, psi2=psi2, omb2=omb2, tdT=tdT,
            WrecT_aug=wrec, WstructT=wstr, WtT=wt, OM=OM, sel=sel,
        ))
    return in_maps


def run(inputs, trace=False):
    if "prog" not in _prog_cache:
        _prog_cache["prog"] = _build_program()
    nc = _prog_cache["prog"]
    in_maps = _pack_inputs(**{k: np.asarray(v) for k, v in inputs.items()
                              if k != "A"})
    res = run_bass_kernel_spmd(
        nc, in_maps, core_ids=list(range(NCORES)), trace=trace
    )
    lam = np.concatenate([res.results[c]["Lam"][:, 0] for c in range(NCORES)])
    zn = np.concatenate(
        [res.results[c]["zn"].reshape(2, BE, H).transpose(1, 0, 2)
         for c in range(NCORES)]
    )
    return (lam.astype(np.float32), zn.astype(np.float32)), res


def kernel(**inputs):
    out, _ = run(inputs, trace=False)
    return out
